# revision 9
# baseline (speedup 1.0000x reference)
"""Trainium2 Bass kernel for a dense transformer layer (attention + FFN).

Sharding: 8 shards = (batch b, sequence half) pairs. Each core computes the
full K/V projections for its batch (2x redundant) and Q/attention/FFN for its
1024-token query slice. No cross-core communication.

On-device layout is feature-major (transposed): activations live as
[feature, token] so every matmul is lhsT.T @ rhs with natural weight layouts.

Precision: projections and the P@V matmul run in fp8e4 DoubleRow (2 k-tiles
per instruction); attention scores run bf16 as concurrent K=64 PE row-tiles;
the FFN runs fp16 (fp8 there costs too much accuracy). Weights for fp8
matmuls are pre-scaled by 64 on the host; the 1/64 is folded into the PSUM
drain op. Residuals fp32.

Structure: attention is sequence-chunk-major — sweep 0 computes K/Q
projections and attention for query chunk 0 (pairs 0..7), sweep 1 runs
attention for chunk 1 while the tensor engine interleaves the O-projection
and FFN of chunk 0 into the gaps (the scalar engine is saturated by Exp).
Softmax normalization is decoupled: raw ctx and row-sums are copied out of
PSUM immediately (releasing the accumulator banks), and the normalize
multiply runs off the critical path.
"""

import numpy as np
import ml_dtypes

B, S, D = 4, 2048, 1024
H, DH, F = 16, 64, 4096
P = 128
NCORES = 8
SQ = B * S // NCORES  # 1024 query tokens per core
DC = D // P  # 8 feature chunks
FC = F // P  # 32 ffn chunks
SKC = S // P  # 16 key chunks
NPAIR = H // 2  # 8 head pairs (2 heads per 128-feature chunk)

BF16 = ml_dtypes.bfloat16
FP8 = ml_dtypes.float8_e4m3
FP16 = np.float16
WS = 64.0  # host-side weight scale for fp8 weights
ATT = DH ** (-0.5)

_CACHE = {}


def _build_program():
    import concourse.mybir as mybir
    import concourse.tile as tile
    from concourse import bacc

    f32 = mybir.dt.float32
    bf16 = mybir.dt.bfloat16
    fp16 = mybir.dt.float16
    fp8 = mybir.dt.float8e4
    AF = mybir.ActivationFunctionType
    ALU = mybir.AluOpType
    DR = mybir.MatmulPerfMode.DoubleRow

    nc = bacc.Bacc("TRN2", target_bir_lowering=False, debug=False, num_devices=NCORES)

    xT_d = nc.dram_tensor("xT", [P, DC, S], fp8, kind="ExternalInput")
    xqT_d = nc.dram_tensor("xqT", [P, DC, SQ], fp8, kind="ExternalInput")
    xres_d = nc.dram_tensor("xres", [P, DC, SQ], f32, kind="ExternalInput")
    wq_d = nc.dram_tensor("wq", [P, DC, D], fp8, kind="ExternalInput")
    wk_d = nc.dram_tensor("wk", [P, DC, D], fp8, kind="ExternalInput")
    wv_d = nc.dram_tensor("wv", [P, DC, D], fp8, kind="ExternalInput")
    wo_d = nc.dram_tensor("wo", [P, DC, D], fp8, kind="ExternalInput")
    w1_d = nc.dram_tensor("w1", [FC, P, DC, P], fp16, kind="ExternalInput")
    w2_d = nc.dram_tensor("w2", [DC, P, FC, P], fp16, kind="ExternalInput")
    bq_d = nc.dram_tensor("bq", [P, DC], f32, kind="ExternalInput")
    bk_d = nc.dram_tensor("bk", [P, DC], f32, kind="ExternalInput")
    bvb_d = nc.dram_tensor("bvb", [P, D], bf16, kind="ExternalInput")
    b1_d = nc.dram_tensor("b1", [P, FC], f32, kind="ExternalInput")
    b2_d = nc.dram_tensor("b2", [P, DC], f32, kind="ExternalInput")
    outT_d = nc.dram_tensor("outT", [P, DC, SQ], f32, kind="ExternalOutput")

    from contextlib import ExitStack

    with tile.TileContext(nc) as tc:
        with ExitStack() as _es:
            _p = lambda name, bufs, **kw: _es.enter_context(
                tc.tile_pool(name=name, bufs=bufs, **kw)
            )
            psA = _p("psA", 2, space="PSUM")
            psS = _p("psS", 2, space="PSUM")
            psC = _p("psC", 2, space="PSUM")
            biasp = _p("biasp", 1)
            ctxp = _p("ctxp", 1)
            wop = _p("wop", 1)
            abp = _p("abp", 1)
            wvp = _p("wvp", 1)
            ws = _p("ws", 3)
            ep = _p("ep", 3)
            ktp = _p("ktp", 1)
            qtp = _p("qtp", 1)
            rp = _p("rp", 6)
            rawp = _p("rawp", 3)
            rbp = _p("rbp", 1)
            ytp = _p("ytp", 1)
            htp = _p("htp", 1)
            w2s = _p("w2s", 2)
            w1s = _p("w1s", 2)
            outp = _p("outp", 2)
            xrp = _p("xrp", 2)
            bq_sb = biasp.tile([P, DC], f32)
            bk_sb = biasp.tile([P, DC], f32)
            b1_sb = biasp.tile([P, FC], f32)
            b2_sb = biasp.tile([P, DC], f32)
            nc.sync.dma_start(bq_sb[:], bq_d[:])
            nc.sync.dma_start(bk_sb[:], bk_d[:])
            nc.sync.dma_start(b1_sb[:], b1_d[:])
            nc.sync.dma_start(b2_sb[:], b2_d[:])

            # normalized ctx (x64) for the O projection, fp8
            ctxT_sb = ctxp.tile([P, DC, SQ], fp8)
            wo_sb = wop.tile([P, DC, D], fp8)

            xTs = [
                abp.tile([P, DC, 512], fp8, tag=f"xT{c}", name=f"xT{c}")
                for c in range(4)
            ]
            wvs = [
                wvp.tile([P, DC, 512], fp8, tag=f"wv{c}", name=f"wv{c}")
                for c in range(2)
            ]
            bvb_sb = abp.tile([P, D], bf16)
            xqT_sb = abp.tile([P, DC, SQ], fp8)
            nc.sync.dma_start(xTs[0][:], xT_d[:, :, 0:512])
            nc.gpsimd.dma_start(wvs[0][:], wv_d[:, :, 0:512])
            nc.sync.dma_start(bvb_sb[:], bvb_d[:])
            for c in range(1, 4):
                nc.sync.dma_start(xTs[c][:], xT_d[:, :, c * 512 : (c + 1) * 512])
            nc.sync.dma_start(wvs[1][:], wv_d[:, :, 512:1024])
            nc.sync.dma_start(xqT_sb[:], xqT_d[:])
            nc.sync.dma_start(wo_sb[:], wo_d[:])

            # V projection, token-major: v[sk, dv] (+ ones column per head)
            v_sb = abp.tile([P, SKC, H, DH + 1], fp8)
            nc.vector.memset(v_sb[:, :, :, DH : DH + 1], 1.0)

            def emit_v(nv, sks, h0=0, h1=8):
                nh = h1 - h0
                for sk in sks:
                    xt = xTs[sk // 4]
                    co = (sk % 4) * P
                    ps = psA.tile([P, 512], f32, tag="ps")
                    for k in range(DC // 2):
                        nc.tensor.matmul(
                            ps[:, : nh * DH],
                            xt[:, 2 * k : 2 * k + 2, co : co + P],
                            wvs[nv][:, 2 * k : 2 * k + 2, h0 * DH : h1 * DH],
                            start=(k == 0),
                            stop=(k == DC // 2 - 1),
                            perf_mode=DR,
                        )
                    nc.vector.scalar_tensor_tensor(
                        v_sb[:, sk, nv * 8 + h0 : nv * 8 + h1, 0:DH],
                        ps[:, : nh * DH].rearrange("p (h d) -> p h d", h=nh),
                        1.0 / WS,
                        bvb_sb[
                            :, nv * 512 + h0 * DH : nv * 512 + h1 * DH
                        ].rearrange("p (h d) -> p h d", h=nh),
                        ALU.mult,
                        ALU.add,
                    )

            # V(nv=1) chunks spread across sweep-0 attention of pairs 1-3
            V1_SPREAD = {1: range(0, 6), 2: range(6, 12), 3: range(12, 16)}

            kts, qts = [], []

            def attn(p, sqn):
                kt, qt = kts[p], qts[p]
                qs = qt[:, sqn * 512 : (sqn + 1) * 512]
                pc0 = psC.tile([P, 512], f32, tag="pc")
                pc1 = psC.tile([P, 512], f32, tag="pc")
                for skp in range(SKC // 2):
                    ED = ep.tile([P, 2, 2, 512], fp8, tag="ed")
                    for half in range(2):
                        sk = 2 * skp + half
                        ss = psS.tile([P, 1024], f32)
                        nc.tensor.matmul(
                            ss[:, 0:512],
                            kt[0:64, sk * P : (sk + 1) * P],
                            qs[0:64],
                            start=True,
                            stop=True,
                        )
                        nc.tensor.matmul(
                            ss[:, 512:1024],
                            kt[64:128, sk * P : (sk + 1) * P],
                            qs[64:128],
                            start=True,
                            stop=True,
                        )
                        nc.scalar.activation(
                            ED[:, :, half, :],
                            ss.rearrange("p (h n) -> p h n", h=2),
                            AF.Exp,
                            scale=ATT,
                        )
                    nc.tensor.matmul(
                        pc0[:65],
                        v_sb[:, 2 * skp : 2 * skp + 2, 2 * p, :],
                        ED[:, 0],
                        start=(skp == 0),
                        stop=(skp == SKC // 2 - 1),
                        perf_mode=DR,
                    )
                    nc.tensor.matmul(
                        pc1[:65],
                        v_sb[:, 2 * skp : 2 * skp + 2, 2 * p + 1, :],
                        ED[:, 1],
                        start=(skp == 0),
                        stop=(skp == SKC // 2 - 1),
                        perf_mode=DR,
                    )
                cols = slice(sqn * 512, (sqn + 1) * 512)
                # drain PSUM fast: raw ctx + row-sums out, normalize later
                rw0 = rawp.tile([64, 512], fp16, tag="raw0")
                rw1 = rawp.tile([64, 512], fp16, tag="raw1")
                s0 = rp.tile([1, 512], f32, tag="s")
                s1 = rp.tile([1, 512], f32, tag="s")
                nc.vector.tensor_copy(rw0[:], pc0[0:64, :])
                nc.vector.tensor_scalar_mul(s0, pc0[64:65, :], 1.0 / WS)
                nc.vector.tensor_copy(rw1[:], pc1[0:64, :])
                nc.vector.tensor_scalar_mul(s1, pc1[64:65, :], 1.0 / WS)
                # off-critical-path normalization: ctx64 = raw * (64/rowsum)
                for hh, s, rw in ((0, s0, rw0), (1, s1, rw1)):
                    r = rp.tile([1, 512], f32, tag="r")
                    nc.vector.reciprocal_approx_fast(r, s)
                    rb = rbp.tile([64, 512], f32, tag="rb")
                    nc.gpsimd.partition_broadcast(rb, r)
                    nc.vector.tensor_mul(
                        ctxT_sb[64 * hh : 64 * hh + 64, p, cols],
                        rw[:],
                        rb,
                    )

            yTs = {}
            accs = {}
            hTs = {}

            def cde_o(sqn):
                # O projection + residual for query chunk sqn
                cols = slice(sqn * 512, (sqn + 1) * 512)
                yT = ytp.tile([P, DC, 512], f32, tag="yt", name=f"yT{sqn}")
                acc = ytp.tile([P, DC, 512], fp16, tag="acc", name=f"acc{sqn}")
                yTs[sqn], accs[sqn] = yT, acc
                for m in range(DC):
                    xr = xrp.tile([P, 512], f32, tag="xr")
                    nc.sync.dma_start(xr[:], xres_d[:, m, cols])
                    ps = psA.tile([P, 512], f32)
                    for k in range(DC // 2):
                        nc.tensor.matmul(
                            ps,
                            wo_sb[:, 2 * k : 2 * k + 2, m * P : (m + 1) * P],
                            ctxT_sb[:, 2 * k : 2 * k + 2, cols],
                            start=(k == 0),
                            stop=(k == DC // 2 - 1),
                            perf_mode=DR,
                        )
                    # y = ps/(64*64) + (x + bo)
                    nc.vector.scalar_tensor_tensor(
                        yT[:, m, :], ps, 1.0 / (WS * WS), xr,
                        ALU.mult, ALU.add,
                    )
                    # fp16(y): FFN input
                    nc.scalar.activation(acc[:, m, :], yT[:, m, :], AF.Copy)

            def cde_f1(sqn, m):
                # FFN layer 1 row-chunk m (fp16) + gelu
                if m == 0:
                    hTs[sqn] = htp.tile([P, FC, 512], fp16, tag="ht", name=f"hT{sqn}")
                w1t = w1s.tile([P, DC, P], fp16, tag="w1c")
                nc.sync.dma_start(w1t[:], w1_d[m])
                ps = psA.tile([P, 512], f32)
                for k in range(DC):
                    nc.tensor.matmul(
                        ps,
                        w1t[:, k, :],
                        accs[sqn][:, k, :],
                        start=(k == 0),
                        stop=(k == DC - 1),
                    )
                nc.scalar.activation(
                    hTs[sqn][:, m, :], ps, AF.Gelu, bias=b1_sb[:, m : m + 1]
                )

            def cde_f2(sqn, m):
                # FFN layer 2 chunk m (fp16) + bias + residual + store
                cols = slice(sqn * 512, (sqn + 1) * 512)
                hw2 = FC // 2
                w2a = w2s.tile([P, hw2, P], fp16, tag="w2c")
                w2b = w2s.tile([P, hw2, P], fp16, tag="w2c")
                nc.sync.dma_start(w2a[:], w2_d[m, :, 0:hw2])
                nc.sync.dma_start(w2b[:], w2_d[m, :, hw2:FC])
                ps = psA.tile([P, 512], f32)
                for k in range(FC):
                    w2t = w2a if k < hw2 else w2b
                    nc.tensor.matmul(
                        ps,
                        w2t[:, k % hw2, :],
                        hTs[sqn][:, k, :],
                        start=(k == 0),
                        stop=(k == FC - 1),
                    )
                ot = outp.tile([P, 512], f32, tag="ot")
                nc.vector.tensor_scalar(
                    ot, ps, b2_sb[:, m : m + 1], None, ALU.add
                )
                nc.vector.tensor_add(ot, ot, yTs[sqn][:, m, :])
                nc.sync.dma_start(outT_d[:, m, cols], ot)

            # ---------------- Sweep 0: projections + attention chunk 0 ------
            emit_v(0, range(SKC))
            for p in range(NPAIR):
                kt = ktp.tile([P, S], fp8, tag=f"kt{p}", name=f"kt{p}")
                qt = qtp.tile([P, SQ], fp8, tag=f"qt{p}", name=f"qt{p}")
                kts.append(kt)
                qts.append(qt)
                wkt = ws.tile([P, DC, P], fp8, tag="wchunk")
                nc.sync.dma_start(wkt[:], wk_d[:, :, p * P : (p + 1) * P])
                for n in range(S // 512):
                    ps = psA.tile([P, 512], f32)
                    for k in range(DC // 2):
                        nc.tensor.matmul(
                            ps,
                            wkt[:, 2 * k : 2 * k + 2, :],
                            xTs[n][:, 2 * k : 2 * k + 2, :],
                            start=(k == 0),
                            stop=(k == DC // 2 - 1),
                            perf_mode=DR,
                        )
                    nc.vector.tensor_scalar(
                        kt[:, n * 512 : (n + 1) * 512],
                        ps, 1.0 / WS, bk_sb[:, p : p + 1],
                        ALU.mult, ALU.add,
                    )
                wqt = ws.tile([P, DC, P], fp8, tag="wchunk")
                nc.sync.dma_start(wqt[:], wq_d[:, :, p * P : (p + 1) * P])
                for n in range(SQ // 512):
                    ps = psA.tile([P, 512], f32)
                    for k in range(DC // 2):
                        nc.tensor.matmul(
                            ps,
                            wqt[:, 2 * k : 2 * k + 2, :],
                            xqT_sb[:, 2 * k : 2 * k + 2, n * 512 : (n + 1) * 512],
                            start=(k == 0),
                            stop=(k == DC // 2 - 1),
                            perf_mode=DR,
                        )
                    nc.vector.tensor_scalar(
                        qt[:, n * 512 : (n + 1) * 512],
                        ps, 1.0 / WS, bq_sb[:, p : p + 1],
                        ALU.mult, ALU.add,
                    )
                attn(p, 0)
                if p in V1_SPREAD:
                    emit_v(1, V1_SPREAD[p])

            # ---------------- Sweep 1: attention chunk 1, FFN chunk 0 -------
            # injected work for chunk 0, one slot per pair of sweep 1
            inject = [
                [("o", 0)],
                [("f1", m) for m in range(0, 5)],
                [("f1", m) for m in range(5, 10)],
                [("f1", m) for m in range(10, 15)],
                [("f1", m) for m in range(15, 20)],
                [("f1", m) for m in range(20, 26)],
                [("f1", m) for m in range(26, 32)],
                [("f2", 0), ("f2", 1)],
            ]
            for p in range(NPAIR):
                attn(p, 1)
                for kind, m in inject[p]:
                    if kind == "o":
                        cde_o(0)
                    elif kind == "f1":
                        cde_f1(0, m)
                    else:
                        cde_f2(0, m)

            # ---------------- Tail: rest of FFN chunk 0 + all of chunk 1 ----
            for m in range(2, DC):
                cde_f2(0, m)
            cde_o(1)
            for m in range(FC):
                cde_f1(1, m)
            for m in range(DC):
                cde_f2(1, m)

    nc.compile()
    return nc


def _get_program():
    if "nc" not in _CACHE:
        _CACHE["nc"] = _build_program()
    return _CACHE["nc"]


def _wlayout(W):
    # [D_in, D_out] -> [P, D_in//P, D_out]
    return np.ascontiguousarray(
        W.reshape(W.shape[0] // P, P, W.shape[1]).transpose(1, 0, 2)
    )


def _blayout(b):
    # [D] -> [P, D//P]
    return np.ascontiguousarray(b.reshape(b.shape[0] // P, P).T)


def prepare_in_maps(x, Wq, bq, Wk, bk, Wv, bv, Wo, bo, W1, b1, W2, b2):
    x = np.asarray(x, np.float32)
    Wq = np.asarray(Wq, np.float32)
    bq = np.asarray(bq, np.float32)
    Wk = np.asarray(Wk, np.float32)
    bk = np.asarray(bk, np.float32)
    Wv = np.asarray(Wv, np.float32)
    bv = np.asarray(bv, np.float32)
    Wo = np.asarray(Wo, np.float32)
    bo = np.asarray(bo, np.float32)
    W1 = np.asarray(W1, np.float32)
    b1 = np.asarray(b1, np.float32)
    W2 = np.asarray(W2, np.float32)
    b2 = np.asarray(b2, np.float32)

    shared = {
        "wq": _wlayout(Wq * WS).astype(FP8),
        "wk": _wlayout(Wk * WS).astype(FP8),
        "wv": _wlayout(Wv * WS).astype(FP8),
        "wo": _wlayout(Wo * WS).astype(FP8),
        "w1": np.ascontiguousarray(
            W1.reshape(DC, P, FC, P).transpose(2, 1, 0, 3)
        ).astype(FP16),
        "w2": np.ascontiguousarray(
            W2.reshape(FC, P, DC, P).transpose(2, 1, 0, 3)
        ).astype(FP16),
        "bq": _blayout(bq),
        "bk": _blayout(bk),
        "bvb": np.ascontiguousarray(np.broadcast_to(bv, (P, D))).astype(BF16),
        "b1": _blayout(b1),
        "b2": _blayout(b2),
    }

    in_maps = []
    for c in range(NCORES):
        b_idx, half = divmod(c, 2)
        xb = x[b_idx]  # [S, D]
        xbT = xb.T  # [D, S]
        xT = np.ascontiguousarray(
            xbT.reshape(DC, P, S).transpose(1, 0, 2)
        ).astype(FP8)
        xqT = np.ascontiguousarray(
            xbT[:, half * SQ : (half + 1) * SQ]
            .reshape(DC, P, SQ)
            .transpose(1, 0, 2)
        ).astype(FP8)
        xres = np.ascontiguousarray(
            (xbT[:, half * SQ : (half + 1) * SQ] + bo[:, None])
            .reshape(DC, P, SQ)
            .transpose(1, 0, 2)
        ).astype(np.float32)
        in_maps.append(dict(shared, xT=xT, xqT=xqT, xres=xres))
    return in_maps


def assemble_out(results):
    out = np.empty((B, S, D), np.float32)
    for c in range(NCORES):
        b_idx, half = divmod(c, 2)
        outT = results[c]["outT"]  # [P, DC, SQ]
        out[b_idx, half * SQ : (half + 1) * SQ] = (
            outT.transpose(1, 0, 2).reshape(D, SQ).T
        )
    return out


def kernel(**inputs):
    from concourse.bass_utils import run_bass_kernel_spmd

    in_maps = prepare_in_maps(**inputs)
    nc = _get_program()
    res = run_bass_kernel_spmd(nc, in_maps, core_ids=list(range(NCORES)))
    return assemble_out(res.results)
